# revision 2
# baseline (speedup 1.0000x reference)
"""Trainium2 Bass kernel v2 for nn_DeepSeekNeuralMLP (SwiGLU MLP with
Catmull-Rom-spline-reconstructed weights), tensor-parallel over 8 NeuronCores.

Changes vs baseline:
  - all big matmuls in bf16 (weights generated into bf16 SBUF tiles,
    hidden_states and the spilled intermediate in bf16)
  - per-core chunk lists permuted kt-major so every matmul lhsT is a
    contiguous [128, 128] slice of the generated weight tiles
  - host premultiplies the u-power rows into the gathered control-point
    taps: the z tensor [21, CPB] is DMA'd directly (no on-device DVE mul)
  - all three weights generated up-front; masked select merges into the
    PSUM tile directly (one DVE + one Act op per block)
"""
import numpy as np
from math import comb

import ml_dtypes
import concourse.bass as bass
from concourse import bacc, tile, mybir
from concourse.bass_utils import run_bass_kernel_spmd

# ----------------------------------------------------------------------------
# static problem geometry (hardcoded; must match the reference)
# ----------------------------------------------------------------------------
HIDDEN = 2048
INTER = 8192
NTOK = 8192                    # 4 * 2048 tokens
NCORES = 8
N = INTER * HIDDEN             # samples per weight (same for all three)
NCTRL = max(16, int(N / 128.9))
NCHUNK = N // 128
CPB = NCHUNK // NCORES         # 16384 chunks per core per weight
IC = INTER // NCORES           # 1024 intermediate rows per core

F32 = mybir.dt.float32
F32R = mybir.dt.float32r
BF16 = mybir.dt.bfloat16
U8 = mybir.dt.uint8

_B_COEF = 0.5 * np.array([
    [0.0, -1.0,  2.0, -1.0],
    [2.0,  0.0, -5.0,  3.0],
    [0.0,  1.0,  4.0, -3.0],
    [0.0,  0.0, -1.0,  1.0],
], dtype=np.float64)           # Catmull-Rom basis b_t(f) coeffs, [tap, power]


def _static_tables():
    t = np.linspace(0.0, NCTRL - 1.0, N, dtype=np.float64)
    i = np.clip(np.floor(t).astype(np.int64), 0, NCTRL - 2)
    k0 = np.arange(NCHUNK, dtype=np.int64) * 128
    j = i[k0]
    iv = i.reshape(NCHUNK, 128)
    m = (iv == j[:, None]).sum(axis=1)
    u = t[k0] - j
    delta = (NCTRL - 1.0) / (N - 1.0)
    return j, u, m, delta


_J, _U, _M, _DELTA = _static_tables()


def _bderiv(y):
    y = np.asarray(y, dtype=np.float64)
    out = np.zeros((4, 4) + y.shape, dtype=np.float64)
    for e in range(4):
        for tp in range(4):
            for p in range(e, 4):
                out[e, tp] += comb(p, e) * _B_COEF[tp, p] * y ** (p - e)
    return out


def _va_vb():
    """Row map: z = (e-1)*5 + tau for e in 1..3 (rows 0..14), z = 15 zero row,
    z = 16 + tau for e = 0 (raw cp taps)."""
    s = np.arange(128, dtype=np.float64)
    dA = _bderiv(s * _DELTA)
    dB = _bderiv(s * _DELTA - 1.0)
    VA = np.zeros((21, 128), dtype=np.float64)
    VB = np.zeros((21, 128), dtype=np.float64)
    for e in range(4):
        for tp in range(4):
            zA = 16 + tp if e == 0 else (e - 1) * 5 + tp
            zB = 16 + (tp + 1) if e == 0 else (e - 1) * 5 + (tp + 1)
            VA[zA] = dA[e, tp]
            VB[zB] = dB[e, tp]
    return VA.astype(np.float32), VB.astype(np.float32)


_VA, _VB = _va_vb()


def _chunklists():
    """Per-core chunk lists in kt-major order.

    gate/up: full weight [8192, 2048] row-major; core r owns rows
      [r*1024, (r+1)*1024).  Local ordered index jp = kt*1024 + i_local
      (kt = h//128), so lhsT tiles for (it, kt) are contiguous columns.
    down: full weight [2048, 8192] row-major; core r owns cols
      [r*1024, (r+1)*1024).  jp = ib*2048 + h (ib = i_local//128).
    """
    jp = np.arange(CPB, dtype=np.int64)
    gu = np.empty((NCORES, CPB), dtype=np.int64)
    dn = np.empty((NCORES, CPB), dtype=np.int64)
    for r in range(NCORES):
        i_local = jp % 1024
        kt = jp // 1024
        gu[r] = r * 16384 + i_local * 16 + kt
        h = jp % 2048
        ib = jp // 2048
        dn[r] = h * 64 + r * 8 + ib
    return gu, dn


_CL_GU, _CL_DN = _chunklists()


def _static_for_clist(cl):
    """cp gather indices [5, CPB], u-power rows [15, CPB], mask [128, CPB]."""
    j = _J[cl]
    u = _U[cl]
    m = _M[cl]
    idx = np.clip(j[None, :] + np.arange(-1, 4)[:, None], 0, NCTRL - 1)
    us = np.zeros((15, cl.size), dtype=np.float64)
    for e in range(1, 4):
        us[(e - 1) * 5:(e - 1) * 5 + 5, :] = (u ** e)[None, :]
    s = np.arange(128, dtype=np.int64)
    mask = (s[:, None] >= m[None, :]).astype(np.uint8)
    return idx, us, np.ascontiguousarray(mask)


_STATIC_GU = [_static_for_clist(_CL_GU[r]) for r in range(NCORES)]
_STATIC_DN = [_static_for_clist(_CL_DN[r]) for r in range(NCORES)]


def _build_z(cp, idx, us):
    """Host-side z tensor [21, CPB]: rows 0..14 = tap*u^e, 15 = 0,
    16..20 = raw taps."""
    taps = np.take(cp, idx)                       # [5, CPB] f32
    z = np.zeros((21, idx.shape[1]), dtype=np.float32)
    z[0:15] = (np.tile(taps, (3, 1)).astype(np.float64) * us).astype(np.float32)
    z[16:21] = taps
    return np.ascontiguousarray(z)


# ----------------------------------------------------------------------------
# device program
# ----------------------------------------------------------------------------
def _emit_gen(nc, pools, z_dram, mask_tile, va_tile, vb_tile, halves):
    """Generate one weight (16384 chunks) into two [128, 8192] bf16 tiles.
    mask_tile is a resident [128, CPB] u8 SBUF tile."""
    for sb in range(16):
        half, lsb = sb // 8, sb % 8
        zp = pools["zp"].tile([21, 1024], F32R, tag="zp")
        # split across two DMA queues to halve per-queue delivery latency
        nc.sync.dma_start(zp[:, 0:512], z_dram[:, sb * 1024:sb * 1024 + 512])
        nc.sync.dma_start(zp[:, 512:1024],
                          z_dram[:, sb * 1024 + 512:(sb + 1) * 1024])
        for blk in range(2):                  # 512-chunk blocks
            zsl = zp[:, blk * 512:(blk + 1) * 512]
            pa = pools["psum"].tile([128, 512], F32, tag="ps")
            pb = pools["psum"].tile([128, 512], F32, tag="ps")
            nc.tensor.matmul(pa[:], va_tile[:], zsl, start=True, stop=True)
            nc.tensor.matmul(pb[:], vb_tile[:], zsl, start=True, stop=True)
            col = (sb * 2 + blk) * 512
            lcol = (lsb * 2 + blk) * 512
            nc.vector.copy_predicated(pa[:], mask_tile[:, col:col + 512], pb[:])
            nc.scalar.copy(halves[half][:, lcol:lcol + 512], pa[:])


def _build_program():
    nc = bacc.Bacc("TRN2", target_bir_lowering=False, debug=False,
                   num_devices=NCORES)

    hsT = nc.dram_tensor("hsT", [HIDDEN, NTOK], BF16, kind="ExternalInput")
    va_d = nc.dram_tensor("va", [21, 128], F32R, kind="ExternalInput")
    vb_d = nc.dram_tensor("vb", [21, 128], F32R, kind="ExternalInput")
    mask_gu_d = nc.dram_tensor("mask_gu", [128, CPB], U8, kind="ExternalInput")
    mask_dn_d = nc.dram_tensor("mask_dn", [128, CPB], U8, kind="ExternalInput")
    z_d = {w: nc.dram_tensor(f"z_{w}", [21, CPB], F32R, kind="ExternalInput")
           for w in ("gate", "up", "down")}
    outT = nc.dram_tensor("outT", [HIDDEN, NTOK], BF16, kind="ExternalOutput")
    inter_d = nc.dram_tensor("inter", [IC, NTOK], BF16, kind="Internal")

    with tile.TileContext(nc) as tc:
        import contextlib
        with contextlib.ExitStack() as ctx:
            pools = {
                "const": ctx.enter_context(tc.tile_pool(name="const", bufs=1)),
                "gen": ctx.enter_context(tc.tile_pool(name="gen", bufs=6)),
                "zp": ctx.enter_context(tc.tile_pool(name="zp", bufs=6)),
                "maskbig": ctx.enter_context(tc.tile_pool(name="maskbig", bufs=2)),
                "hs": ctx.enter_context(tc.tile_pool(name="hs", bufs=32)),
                "sil": ctx.enter_context(tc.tile_pool(name="sil", bufs=3)),
                "inter": ctx.enter_context(tc.tile_pool(name="inter", bufs=3)),
                "out": ctx.enter_context(tc.tile_pool(name="out", bufs=3)),
                "psum": ctx.enter_context(
                    tc.tile_pool(name="psum", bufs=6, space="PSUM")),
            }
            va_t = pools["const"].tile([21, 128], F32R, tag="va")
            vb_t = pools["const"].tile([21, 128], F32R, tag="vb")
            nc.sync.dma_start(va_t[:], va_d[:])
            nc.sync.dma_start(vb_t[:], vb_d[:])

            # resident masks, DMA'd once in 4 column strips each (long
            # contiguous per-partition runs -> efficient descriptors)
            mk_gu = pools["maskbig"].tile([128, CPB], U8, tag="mb", name="mgu")
            mk_dn = pools["maskbig"].tile([128, CPB], U8, tag="mb", name="mdn")
            for s in range(4):
                c = s * (CPB // 4)
                nc.sync.dma_start(mk_gu[:, c:c + CPB // 4],
                                  mask_gu_d[:, c:c + CPB // 4])
            for s in range(4):
                c = s * (CPB // 4)
                nc.sync.dma_start(mk_dn[:, c:c + CPB // 4],
                                  mask_dn_d[:, c:c + CPB // 4])

            # ---- stage A: generate all three weight shards (bf16) ----
            wh = {w: [pools["gen"].tile([128, 8192], BF16, tag="gen",
                                        name=f"{w}_h{i}") for i in range(2)]
                  for w in ("gate", "up", "down")}
            _emit_gen(nc, pools, z_d["gate"], mk_gu, va_t, vb_t, wh["gate"])
            _emit_gen(nc, pools, z_d["up"], mk_gu, va_t, vb_t, wh["up"])
            _emit_gen(nc, pools, z_d["down"], mk_dn, va_t, vb_t, wh["down"])

            # ---- stage B: gate/up matmuls + SwiGLU, spill inter to DRAM ----
            # lhsT for (it, kt): wh[w][kt//8][:, (kt%8)*1024 + it*128 :+128]
            for tb in range(16):
                hs_tiles = []
                for kt in range(16):
                    t = pools["hs"].tile([128, 512], BF16, tag="t")
                    nc.sync.dma_start(
                        t[:], hsT[kt * 128:(kt + 1) * 128, tb * 512:(tb + 1) * 512])
                    hs_tiles.append(t)
                for it in range(8):
                    pg = pools["psum"].tile([128, 512], F32, tag="ps")
                    pu = pools["psum"].tile([128, 512], F32, tag="ps")
                    for kt in range(16):
                        half, lkt = kt // 8, kt % 8
                        base = lkt * 1024 + it * 128
                        lg = wh["gate"][half][:, base:base + 128]
                        lu = wh["up"][half][:, base:base + 128]
                        rhs = hs_tiles[kt][:]
                        nc.tensor.matmul(pg[:], lg, rhs,
                                         start=(kt == 0), stop=(kt == 15))
                        nc.tensor.matmul(pu[:], lu, rhs,
                                         start=(kt == 0), stop=(kt == 15))
                    sil = pools["sil"].tile([128, 512], F32, tag="sil")
                    nc.scalar.activation(sil[:], pg[:],
                                         mybir.ActivationFunctionType.Silu)
                    itile = pools["inter"].tile([128, 512], BF16, tag="itile")
                    nc.vector.tensor_mul(itile[:], sil[:], pu[:])
                    nc.sync.dma_start(
                        inter_d[it * 128:(it + 1) * 128, tb * 512:(tb + 1) * 512],
                        itile[:])

            # ---- stage D: down matmul, partial out [h, t] ----
            # lhsT for (ht, ib): wh["down"][ib//4][:, (ib%4)*2048 + ht*128 :+128]
            for tb in range(16):
                int_tiles = []
                for ib in range(8):
                    t = pools["hs"].tile([128, 512], BF16, tag="t")
                    nc.sync.dma_start(
                        t[:], inter_d[ib * 128:(ib + 1) * 128, tb * 512:(tb + 1) * 512])
                    int_tiles.append(t)
                for ht in range(16):
                    pd = pools["psum"].tile([128, 512], F32, tag="ps")
                    for ib in range(8):
                        half, lib = ib // 4, ib % 4
                        base = lib * 2048 + ht * 128
                        ld = wh["down"][half][:, base:base + 128]
                        nc.tensor.matmul(pd[:], ld, int_tiles[ib][:],
                                         start=(ib == 0), stop=(ib == 7))
                    ot = pools["out"].tile([128, 512], BF16, tag="ot")
                    nc.scalar.copy(ot[:], pd[:])
                    c0 = tb * 512
                    nc.sync.dma_start(
                        outT[ht * 128:(ht + 1) * 128, c0:c0 + 256],
                        ot[:, 0:256])
                    nc.sync.dma_start(
                        outT[ht * 128:(ht + 1) * 128, c0 + 256:c0 + 512],
                        ot[:, 256:512])

    nc.compile()
    return nc


_NC_CACHE = None


def _get_program():
    global _NC_CACHE
    if _NC_CACHE is None:
        _NC_CACHE = _build_program()
    return _NC_CACHE


def _in_maps(hidden_states, gate_cp, up_cp, down_cp):
    hs = np.asarray(hidden_states, dtype=np.float32).reshape(NTOK, HIDDEN).T
    hs_bf = np.ascontiguousarray(hs).astype(ml_dtypes.bfloat16)
    cps = {"gate": np.asarray(gate_cp, dtype=np.float32),
           "up": np.asarray(up_cp, dtype=np.float32),
           "down": np.asarray(down_cp, dtype=np.float32)}
    maps = []
    for r in range(NCORES):
        idx_gu, us_gu, mask_gu = _STATIC_GU[r]
        idx_dn, us_dn, mask_dn = _STATIC_DN[r]
        m = {"hsT": hs_bf, "va": _VA, "vb": _VB,
             "mask_gu": mask_gu, "mask_dn": mask_dn}
        for w in ("gate", "up", "down"):
            idx = idx_gu if w in ("gate", "up") else idx_dn
            us = us_gu if w in ("gate", "up") else us_dn
            m[f"z_{w}"] = _build_z(cps[w], idx, us)
        maps.append(m)
    return maps


def kernel(hidden_states, gate_cp, up_cp, down_cp, _trace=False):
    nc = _get_program()
    maps = _in_maps(hidden_states, gate_cp, up_cp, down_cp)
    res = run_bass_kernel_spmd(nc, maps, core_ids=list(range(NCORES)),
                               trace=_trace)
    out_T = np.zeros((HIDDEN, NTOK), dtype=np.float32)
    for r in range(NCORES):
        out_T += np.asarray(res.results[r]["outT"], dtype=np.float32)
    out = np.ascontiguousarray(out_T.T).reshape(4, 2048, HIDDEN)
    if _trace:
        kernel.last_results = res
    return out


# revision 3
# speedup vs baseline: 1.2174x; 1.2174x over previous
"""Trainium2 Bass kernel v2 for nn_DeepSeekNeuralMLP (SwiGLU MLP with
Catmull-Rom-spline-reconstructed weights), tensor-parallel over 8 NeuronCores.

Changes vs baseline:
  - all big matmuls in bf16 (weights generated into bf16 SBUF tiles,
    hidden_states and the spilled intermediate in bf16)
  - per-core chunk lists permuted kt-major so every matmul lhsT is a
    contiguous [128, 128] slice of the generated weight tiles
  - host premultiplies the u-power rows into the gathered control-point
    taps: the z tensor [21, CPB] is DMA'd directly (no on-device DVE mul)
  - all three weights generated up-front; masked select merges into the
    PSUM tile directly (one DVE + one Act op per block)
"""
import numpy as np
from math import comb

import ml_dtypes
import concourse.bass as bass
from concourse import bacc, tile, mybir
from concourse.bass_utils import run_bass_kernel_spmd

# ----------------------------------------------------------------------------
# static problem geometry (hardcoded; must match the reference)
# ----------------------------------------------------------------------------
HIDDEN = 2048
INTER = 8192
NTOK = 8192                    # 4 * 2048 tokens
NCORES = 8
N = INTER * HIDDEN             # samples per weight (same for all three)
NCTRL = max(16, int(N / 128.9))
NCHUNK = N // 128
CPB = NCHUNK // NCORES         # 16384 chunks per core per weight
IC = INTER // NCORES           # 1024 intermediate rows per core

F32 = mybir.dt.float32
F32R = mybir.dt.float32r
BF16 = mybir.dt.bfloat16
U8 = mybir.dt.uint8

_B_COEF = 0.5 * np.array([
    [0.0, -1.0,  2.0, -1.0],
    [2.0,  0.0, -5.0,  3.0],
    [0.0,  1.0,  4.0, -3.0],
    [0.0,  0.0, -1.0,  1.0],
], dtype=np.float64)           # Catmull-Rom basis b_t(f) coeffs, [tap, power]


def _static_tables():
    t = np.linspace(0.0, NCTRL - 1.0, N, dtype=np.float64)
    i = np.clip(np.floor(t).astype(np.int64), 0, NCTRL - 2)
    k0 = np.arange(NCHUNK, dtype=np.int64) * 128
    j = i[k0]
    iv = i.reshape(NCHUNK, 128)
    m = (iv == j[:, None]).sum(axis=1)
    u = t[k0] - j
    delta = (NCTRL - 1.0) / (N - 1.0)
    return j, u, m, delta


_J, _U, _M, _DELTA = _static_tables()


def _bderiv(y):
    y = np.asarray(y, dtype=np.float64)
    out = np.zeros((4, 4) + y.shape, dtype=np.float64)
    for e in range(4):
        for tp in range(4):
            for p in range(e, 4):
                out[e, tp] += comb(p, e) * _B_COEF[tp, p] * y ** (p - e)
    return out


def _va_vb():
    """Row map: z = (e-1)*5 + tau for e in 1..3 (rows 0..14), z = 15 zero row,
    z = 16 + tau for e = 0 (raw cp taps)."""
    s = np.arange(128, dtype=np.float64)
    dA = _bderiv(s * _DELTA)
    dB = _bderiv(s * _DELTA - 1.0)
    VA = np.zeros((21, 128), dtype=np.float64)
    VB = np.zeros((21, 128), dtype=np.float64)
    for e in range(4):
        for tp in range(4):
            zA = 16 + tp if e == 0 else (e - 1) * 5 + tp
            zB = 16 + (tp + 1) if e == 0 else (e - 1) * 5 + (tp + 1)
            VA[zA] = dA[e, tp]
            VB[zB] = dB[e, tp]
    return VA.astype(np.float32), VB.astype(np.float32)


_VA, _VB = _va_vb()


def _chunklists():
    """Per-core chunk lists in kt-major order.

    gate/up: full weight [8192, 2048] row-major; core r owns rows
      [r*1024, (r+1)*1024).  Local ordered index jp = kt*1024 + i_local
      (kt = h//128), so lhsT tiles for (it, kt) are contiguous columns.
    down: full weight [2048, 8192] row-major; core r owns cols
      [r*1024, (r+1)*1024).  jp = ib*2048 + h (ib = i_local//128).
    """
    jp = np.arange(CPB, dtype=np.int64)
    gu = np.empty((NCORES, CPB), dtype=np.int64)
    dn = np.empty((NCORES, CPB), dtype=np.int64)
    for r in range(NCORES):
        i_local = jp % 1024
        kt = jp // 1024
        gu[r] = r * 16384 + i_local * 16 + kt
        h = jp % 2048
        ib = jp // 2048
        dn[r] = h * 64 + r * 8 + ib
    return gu, dn


_CL_GU, _CL_DN = _chunklists()


def _static_for_clist(cl):
    """cp gather indices [5, CPB], u-power rows [15, CPB], mask [128, CPB]."""
    j = _J[cl]
    u = _U[cl]
    m = _M[cl]
    idx = np.clip(j[None, :] + np.arange(-1, 4)[:, None], 0, NCTRL - 1)
    us = np.zeros((15, cl.size), dtype=np.float64)
    for e in range(1, 4):
        us[(e - 1) * 5:(e - 1) * 5 + 5, :] = (u ** e)[None, :]
    s = np.arange(128, dtype=np.int64)
    mask = (s[:, None] >= m[None, :]).astype(np.uint8)
    return idx, us, np.ascontiguousarray(mask)


_STATIC_GU = [_static_for_clist(_CL_GU[r]) for r in range(NCORES)]
_STATIC_DN = [_static_for_clist(_CL_DN[r]) for r in range(NCORES)]


def _build_z(cp, idx, us):
    """Host-side z tensor [21, CPB]: rows 0..14 = tap*u^e, 15 = 0,
    16..20 = raw taps."""
    taps = np.take(cp, idx)                       # [5, CPB] f32
    z = np.zeros((21, idx.shape[1]), dtype=np.float32)
    z[0:15] = (np.tile(taps, (3, 1)).astype(np.float64) * us).astype(np.float32)
    z[16:21] = taps
    return np.ascontiguousarray(z)


# ----------------------------------------------------------------------------
# device program
# ----------------------------------------------------------------------------
def _emit_gen(nc, pools, z_dram, mask_tile, va_tile, vb_tile, halves,
              pre=None):
    """Generate one weight (16384 chunks) into two [128, 8192] bf16 tiles.
    mask_tile is a resident [128, CPB] u8 SBUF tile.  pre maps sb -> an
    already-DMA'd zp tile (startup prefetch)."""
    for sb in range(16):
        half, lsb = sb // 8, sb % 8
        if pre is not None and sb in pre:
            zp = pre[sb]
        else:
            zp = pools["zp"].tile([21, 1024], F32R, tag="zp")
            # split across two DMA queues to halve per-queue latency
            nc.sync.dma_start(zp[:, 0:512],
                              z_dram[:, sb * 1024:sb * 1024 + 512])
            nc.sync.dma_start(zp[:, 512:1024],
                              z_dram[:, sb * 1024 + 512:(sb + 1) * 1024])
        for blk in range(2):                  # 512-chunk blocks
            zsl = zp[:, blk * 512:(blk + 1) * 512]
            pa = pools["psum"].tile([128, 512], F32, tag="ps")
            pb = pools["psum"].tile([128, 512], F32, tag="ps")
            nc.tensor.matmul(pa[:], va_tile[:], zsl, start=True, stop=True)
            nc.tensor.matmul(pb[:], vb_tile[:], zsl, start=True, stop=True)
            col = (sb * 2 + blk) * 512
            lcol = (lsb * 2 + blk) * 512
            nc.vector.copy_predicated(pa[:], mask_tile[:, col:col + 512], pb[:])
            nc.scalar.copy(halves[half][:, lcol:lcol + 512], pa[:])


def _build_program():
    nc = bacc.Bacc("TRN2", target_bir_lowering=False, debug=False,
                   num_devices=NCORES)

    hsT = nc.dram_tensor("hsT", [HIDDEN, NTOK], BF16, kind="ExternalInput")
    va_d = nc.dram_tensor("va", [21, 128], F32R, kind="ExternalInput")
    vb_d = nc.dram_tensor("vb", [21, 128], F32R, kind="ExternalInput")
    mask_gu_d = nc.dram_tensor("mask_gu", [128, CPB], U8, kind="ExternalInput")
    mask_dn_d = nc.dram_tensor("mask_dn", [128, CPB], U8, kind="ExternalInput")
    z_d = {w: nc.dram_tensor(f"z_{w}", [21, CPB], F32R, kind="ExternalInput")
           for w in ("gate", "up", "down")}
    outT = nc.dram_tensor("outT", [HIDDEN, NTOK], BF16, kind="ExternalOutput")
    inter_d = nc.dram_tensor("inter", [IC, NTOK], BF16, kind="Internal")

    with tile.TileContext(nc) as tc:
        import contextlib
        with contextlib.ExitStack() as ctx:
            pools = {
                "const": ctx.enter_context(tc.tile_pool(name="const", bufs=1)),
                "gen": ctx.enter_context(tc.tile_pool(name="gen", bufs=6)),
                "zp": ctx.enter_context(tc.tile_pool(name="zp", bufs=6)),
                "maskbig": ctx.enter_context(tc.tile_pool(name="maskbig", bufs=2)),
                "hs": ctx.enter_context(tc.tile_pool(name="hs", bufs=32)),
                "sil": ctx.enter_context(tc.tile_pool(name="sil", bufs=3)),
                "inter": ctx.enter_context(tc.tile_pool(name="inter", bufs=3)),
                "out": ctx.enter_context(tc.tile_pool(name="out", bufs=3)),
                "psum": ctx.enter_context(
                    tc.tile_pool(name="psum", bufs=6, space="PSUM")),
            }
            va_t = pools["const"].tile([21, 128], F32R, tag="va")
            vb_t = pools["const"].tile([21, 128], F32R, tag="vb")
            nc.sync.dma_start(va_t[:], va_d[:])
            nc.sync.dma_start(vb_t[:], vb_d[:])

            # first z blocks ahead of the mask strips so the opening matmuls
            # aren't queued behind them
            zpre = {}
            for sb in range(2):
                zp = pools["zp"].tile([21, 1024], F32R, tag="zp",
                                      name=f"zpre{sb}")
                nc.sync.dma_start(zp[:, 0:512],
                                  z_d["gate"][:, sb * 1024:sb * 1024 + 512])
                nc.sync.dma_start(zp[:, 512:1024],
                                  z_d["gate"][:, sb * 1024 + 512:(sb + 1) * 1024])
                zpre[sb] = zp

            # resident masks; narrow leading strips so the first merges are
            # not stuck behind a single fat transfer
            mk_gu = pools["maskbig"].tile([128, CPB], U8, tag="mb", name="mgu")
            mk_dn = pools["maskbig"].tile([128, CPB], U8, tag="mb", name="mdn")
            strips = [512, 512, 1024, 2048, 4096, 8192]
            c = 0
            for w_ in strips:
                nc.sync.dma_start(mk_gu[:, c:c + w_], mask_gu_d[:, c:c + w_])
                c += w_
            c = 0
            for w_ in strips:
                nc.sync.dma_start(mk_dn[:, c:c + w_], mask_dn_d[:, c:c + w_])
                c += w_

            # ---- stage A: generate all three weight shards (bf16) ----
            wh = {w: [pools["gen"].tile([128, 8192], BF16, tag="gen",
                                        name=f"{w}_h{i}") for i in range(2)]
                  for w in ("gate", "up", "down")}
            _emit_gen(nc, pools, z_d["gate"], mk_gu, va_t, vb_t, wh["gate"],
                      pre=zpre)
            _emit_gen(nc, pools, z_d["up"], mk_gu, va_t, vb_t, wh["up"])
            _emit_gen(nc, pools, z_d["down"], mk_dn, va_t, vb_t, wh["down"])

            # ---- stage B: gate/up matmuls + SwiGLU, spill inter to DRAM ----
            # lhsT for (it, kt): wh[w][kt//8][:, (kt%8)*1024 + it*128 :+128]
            for tb in range(16):
                hs_tiles = []
                for kt in range(16):
                    t = pools["hs"].tile([128, 512], BF16, tag="t")
                    nc.sync.dma_start(
                        t[:], hsT[kt * 128:(kt + 1) * 128, tb * 512:(tb + 1) * 512])
                    hs_tiles.append(t)
                for it in range(8):
                    pg = pools["psum"].tile([128, 512], F32, tag="ps")
                    pu = pools["psum"].tile([128, 512], F32, tag="ps")
                    for kt in range(16):
                        half, lkt = kt // 8, kt % 8
                        base = lkt * 1024 + it * 128
                        lg = wh["gate"][half][:, base:base + 128]
                        lu = wh["up"][half][:, base:base + 128]
                        rhs = hs_tiles[kt][:]
                        nc.tensor.matmul(pg[:], lg, rhs,
                                         start=(kt == 0), stop=(kt == 15))
                        nc.tensor.matmul(pu[:], lu, rhs,
                                         start=(kt == 0), stop=(kt == 15))
                    sil = pools["sil"].tile([128, 512], F32, tag="sil")
                    nc.scalar.activation(sil[:], pg[:],
                                         mybir.ActivationFunctionType.Silu)
                    itile = pools["inter"].tile([128, 512], BF16, tag="itile")
                    nc.vector.tensor_mul(itile[:], sil[:], pu[:])
                    nc.sync.dma_start(
                        inter_d[it * 128:(it + 1) * 128, tb * 512:(tb + 1) * 512],
                        itile[:])

            # ---- stage D: down matmul, partial out [h, t] ----
            # lhsT for (ht, ib): wh["down"][ib//4][:, (ib%4)*2048 + ht*128 :+128]
            for tb in range(16):
                int_tiles = []
                for ib in range(8):
                    t = pools["hs"].tile([128, 512], BF16, tag="t")
                    nc.sync.dma_start(
                        t[:], inter_d[ib * 128:(ib + 1) * 128, tb * 512:(tb + 1) * 512])
                    int_tiles.append(t)
                for ht in range(16):
                    pd = pools["psum"].tile([128, 512], F32, tag="ps")
                    for ib in range(8):
                        half, lib = ib // 4, ib % 4
                        base = lib * 2048 + ht * 128
                        ld = wh["down"][half][:, base:base + 128]
                        nc.tensor.matmul(pd[:], ld, int_tiles[ib][:],
                                         start=(ib == 0), stop=(ib == 7))
                    ot = pools["out"].tile([128, 512], BF16, tag="ot")
                    nc.scalar.copy(ot[:], pd[:])
                    c0 = tb * 512
                    nc.sync.dma_start(
                        outT[ht * 128:(ht + 1) * 128, c0:c0 + 256],
                        ot[:, 0:256])
                    nc.sync.dma_start(
                        outT[ht * 128:(ht + 1) * 128, c0 + 256:c0 + 512],
                        ot[:, 256:512])

    nc.compile()
    return nc


_NC_CACHE = None


def _get_program():
    global _NC_CACHE
    if _NC_CACHE is None:
        _NC_CACHE = _build_program()
    return _NC_CACHE


def _in_maps(hidden_states, gate_cp, up_cp, down_cp):
    hs = np.asarray(hidden_states, dtype=np.float32).reshape(NTOK, HIDDEN).T
    hs_bf = np.ascontiguousarray(hs).astype(ml_dtypes.bfloat16)
    cps = {"gate": np.asarray(gate_cp, dtype=np.float32),
           "up": np.asarray(up_cp, dtype=np.float32),
           "down": np.asarray(down_cp, dtype=np.float32)}
    maps = []
    for r in range(NCORES):
        idx_gu, us_gu, mask_gu = _STATIC_GU[r]
        idx_dn, us_dn, mask_dn = _STATIC_DN[r]
        m = {"hsT": hs_bf, "va": _VA, "vb": _VB,
             "mask_gu": mask_gu, "mask_dn": mask_dn}
        for w in ("gate", "up", "down"):
            idx = idx_gu if w in ("gate", "up") else idx_dn
            us = us_gu if w in ("gate", "up") else us_dn
            m[f"z_{w}"] = _build_z(cps[w], idx, us)
        maps.append(m)
    return maps


def kernel(hidden_states, gate_cp, up_cp, down_cp, _trace=False):
    nc = _get_program()
    maps = _in_maps(hidden_states, gate_cp, up_cp, down_cp)
    res = run_bass_kernel_spmd(nc, maps, core_ids=list(range(NCORES)),
                               trace=_trace)
    out_T = np.zeros((HIDDEN, NTOK), dtype=np.float32)
    for r in range(NCORES):
        out_T += np.asarray(res.results[r]["outT"], dtype=np.float32)
    out = np.ascontiguousarray(out_T.T).reshape(4, 2048, HIDDEN)
    if _trace:
        kernel.last_results = res
    return out
